# revision 27
# baseline (speedup 1.0000x reference)
"""Trainium2 Bass kernel for the Context Encoder problem:

    ce  = c2e_weight[nodes]            # [N, 128] embedding gather
    h   = relu(ce @ w1.T + b1)         # [N, 128]
    out = relu(h @ w2.T + b2)          # [N, 128]

Strategy (8 NeuronCores, unique-row compaction, bf16):
  200000 node ids cover ~86.4% of the 100000-row vocab, so transforming
  the table rows that are actually referenced is less total work than
  gathering per-node rows (the output of a per-node gather would be 2.3x
  larger than the compacted table).

  - Host computes used = unique(nodes) (~86.4k rows), pads to a fixed
    8*10880 = 87040 rows, gathers those table rows, downcasts to bf16
    and hands core i its contiguous d-major window [128, 10880].
  - The MLP weights and biases ride as a 258-column bf16 prefix of the
    same stream (separate [128, small] constant DMAs cost ~6us: 128
    tiny descriptors each paying the ~0.8us HBM round trip).
  - All input chunk DMAs are issued up-front on the sync HWDGE ring so
    the reads stream at full bandwidth, decoupled from compute; output
    DMAs follow on the same ring (no head-of-line risk once every
    input is already queued).
  - Each core streams its window through the 2-layer MLP: matmuls in
    bf16 (1 cycle/row on PE vs 4 for fp32), fp32 PSUM accumulation,
    relu+bias fused on ACT/DVE draining PSUM, bf16 results DMA'd out.
    Within a chunk the two layers run as phases; matmuls are 512 wide
    (one fp32 PSUM bank), relu+bias ops 1024 wide over double-buffered
    2-bank PSUM tiles, and the two relu stages alternate between ACT
    and DVE by chunk parity for balance.
  - Host upcasts and applies out = T2used[inverse(nodes)] as the
    unshard step.  If a pathological input references more than 87040
    distinct rows, a full-vocab variant (98 blocks/core, staggered
    windows) is built and used instead.
"""

import os
import sys

for _p in ("/opt/trn_rl_repo",):
    if _p not in sys.path:
        sys.path.insert(0, _p)

import numpy as np
import ml_dtypes

import concourse.bass as bass
import concourse.mybir as mybir
import concourse.tile as tile
from concourse import bacc
from concourse.bass_utils import run_bass_kernel_spmd
from concourse.tile import TileContext

P = 128
D = 128
N_CORES = 8
VOCAB = 100000
BLOCKS = 85                # vocab blocks (of 128 rows) per core, compact path
FULL_BLOCKS = 98           # fallback: cover the whole vocab with overlap
CHUNK = 2048               # columns per input chunk / out DMA
RELU_W = 1024              # columns per relu op (2 fp32 PSUM banks)
FW = 512                   # matmul free width (one fp32 PSUM bank)
PREFIX = 2 * D + 64        # packed w1t | w2t | b1 | b2 | pad ahead of the
                           # data (pad keeps the data start 128B-aligned)

BF16 = ml_dtypes.bfloat16


def build_nc(blocks: int):
    f32 = mybir.dt.float32
    bf16 = mybir.dt.bfloat16
    nc = bacc.Bacc("TRN2", target_bir_lowering=False, debug=False,
                   num_devices=N_CORES)

    rows = blocks * P
    tsl_t = nc.dram_tensor("tslice", [P, PREFIX + rows], bf16,
                           kind="ExternalInput").ap()
    out_t = nc.dram_tensor("out", [P, rows], bf16,
                           kind="ExternalOutput").ap()

    # data-column chunks: small first chunk so compute starts early
    chunks = [RELU_W]
    while sum(chunks) + CHUNK <= rows:
        chunks.append(CHUNK)
    rem = rows - sum(chunks)
    assert rem % P == 0
    if rem:
        chunks.append(rem)

    def pieces(cb, w):
        off = 0
        while off < cb:
            yield off, min(w, cb - off)
            off += w

    with TileContext(nc) as tc:
        with (
            tc.tile_pool(name="const", bufs=1) as cpool,
            tc.tile_pool(name="win0", bufs=1) as g0pool,
            tc.tile_pool(name="win", bufs=len(chunks) - 1) as gpool,
            tc.tile_pool(name="work", bufs=3) as wpool,
            tc.tile_pool(name="obuf", bufs=3) as opool,
            tc.tile_pool(name="psum", bufs=2, space="PSUM") as ppool,
        ):
            # PE p-state warmup: the HAM throttle only unlocks 2.4 GHz after
            # ~3.4us of sustained activity, and the PE would otherwise idle
            # for the whole DMA-issue window.  Run throwaway matmuls on a
            # memset tile (and one activation, which also hoists the ACT
            # table load) while the input stream is still in flight.
            dummy = cpool.tile([P, FW], bf16, tag="warm")
            nc.gpsimd.memset(dummy[:], 0.0)
            sink = wpool.tile([P, RELU_W], bf16, tag="hT")
            for wi in range(2):
                warm_ps = ppool.tile([P, RELU_W], f32, tag="h")
                for off in (0, FW):
                    nc.tensor.matmul(out=warm_ps[:, off:off + FW],
                                     lhsT=dummy[:, 0:D], rhs=dummy[:],
                                     start=True, stop=True)
                if wi == 0:
                    nc.scalar.activation(sink[:], warm_ps[:],
                                         mybir.ActivationFunctionType.Relu)

            # issue every input chunk DMA up-front: the sync ring streams
            # the whole window (constants prefix included) at full rate
            win0 = g0pool.tile([P, PREFIX + chunks[0]], bf16, tag="win0")
            nc.sync.dma_start(out=win0[:], in_=tsl_t[:, :PREFIX + chunks[0]])
            wins = [(win0, PREFIX)]
            r0 = PREFIX + chunks[0]
            for cb in chunks[1:]:
                win = gpool.tile([P, CHUNK], bf16, tag="win")
                nc.sync.dma_start(out=win[:, :cb], in_=tsl_t[:, r0:r0 + cb])
                wins.append((win, 0))
                r0 += cb

            w1t_sb = win0[:, 0:D]
            w2t_sb = win0[:, D:2 * D]
            # stage the biases to fp32 once (GpSimd is otherwise idle);
            # ACT/DVE then read a plain fp32 per-partition scalar
            bias_f32 = cpool.tile([P, 2], f32, tag="bias")
            nc.gpsimd.tensor_scalar_add(
                out=bias_f32[:], in0=win0[:, 2 * D:2 * D + 2], scalar1=0.0)
            b1_sb = bias_f32[:, 0:1]
            b2_sb = bias_f32[:, 1:2]

            def relu_bias(out_ap, in_ap, bias_ap, on_act: bool):
                if on_act:
                    nc.scalar.activation(out_ap, in_ap,
                                         mybir.ActivationFunctionType.Relu,
                                         bias=bias_ap)
                else:
                    nc.vector.tensor_scalar(
                        out=out_ap, in0=in_ap, scalar1=bias_ap,
                        scalar2=0.0, op0=mybir.AluOpType.add,
                        op1=mybir.AluOpType.max)

            r0 = 0
            for ci, cb in enumerate(chunks):
                win, base = wins[ci]
                # fixed engine assignment (relu1=ACT, relu2=DVE): alternating
                # by chunk parity queues a chunk's relu1 behind the previous
                # chunk's relu2 in the same engine FIFO, serializing chunks
                par = True

                # layer 1 phase: relu per RELU_W group
                hts = []
                for goff, gw in pieces(cb, RELU_W):
                    h_ps = ppool.tile([P, RELU_W], f32, tag="h")
                    for off, w in pieces(gw, FW):
                        nc.tensor.matmul(
                            out=h_ps[:, off:off + w],
                            lhsT=w1t_sb,
                            rhs=win[:, base + goff + off:base + goff + off + w],
                            start=True, stop=True)
                    hT = wpool.tile([P, RELU_W], bf16, tag="hT")
                    relu_bias(hT[:, :gw], h_ps[:, :gw], b1_sb, par)
                    hts.append((hT, gw))

                # layer 2 phase
                ob = opool.tile([P, CHUNK], bf16, tag="ob")
                for (hT, gw), (goff, _) in zip(hts, pieces(cb, RELU_W)):
                    o_ps = ppool.tile([P, RELU_W], f32, tag="o")
                    for off, w in pieces(gw, FW):
                        nc.tensor.matmul(
                            out=o_ps[:, off:off + w],
                            lhsT=w2t_sb,
                            rhs=hT[:, off:off + w],
                            start=True, stop=True)
                    relu_bias(ob[:, goff:goff + gw], o_ps[:, :gw], b2_sb,
                              not par)

                nc.sync.dma_start(out=out_t[:, r0:r0 + cb], in_=ob[:, :cb])
                r0 += cb

    nc.compile()
    # NOTE: stripping the redundant per-matmul InstLdweights (same weights
    # reloaded within a phase) looked like ~2-3us of PE time, but executing
    # non-self-loading matmuls without an adjacent weight load hard-crashes
    # the exec unit on TRN2 (NRT_EXEC_UNIT_UNRECOVERABLE) — don't.
    return nc


_CACHED_NC = {}
LAST_RESULTS = None


def _get_nc(blocks: int):
    nc = _CACHED_NC.get(blocks)
    if nc is None:
        nc = _CACHED_NC[blocks] = build_nc(blocks)
    return nc


def _run(nc, in_maps):
    global LAST_RESULTS
    trace = os.environ.get("BASS_KERNEL_TRACE") == "1"
    if trace:
        try:  # tracing needs the NTFF hook; degrade silently without it
            import antenv.axon_hooks  # noqa: F401
        except ImportError:
            trace = False
    res = run_bass_kernel_spmd(nc, in_maps, core_ids=list(range(N_CORES)),
                               trace=trace)
    LAST_RESULTS = res
    return res


def _prefix_block(w1, b1, w2, b2):
    w1t = np.ascontiguousarray(np.asarray(w1, dtype=np.float32).T)
    w2t = np.ascontiguousarray(np.asarray(w2, dtype=np.float32).T)
    b1c = np.asarray(b1, dtype=np.float32).reshape(P, 1)
    b2c = np.asarray(b2, dtype=np.float32).reshape(P, 1)
    pad = np.zeros((P, PREFIX - 2 * D - 2), dtype=np.float32)
    return np.concatenate([w1t, w2t, b1c, b2c, pad], axis=1).astype(BF16)


def kernel(nodes, c2e_weight, w1, b1, w2, b2):
    nodes = np.asarray(nodes)
    c2e_weight = np.asarray(c2e_weight, dtype=np.float32)
    prefix = _prefix_block(w1, b1, w2, b2)            # [128, PREFIX] bf16

    vocab = c2e_weight.shape[0]
    assert vocab == VOCAB, vocab

    used, inv = np.unique(nodes, return_inverse=True)
    tot = BLOCKS * P * N_CORES
    rows = BLOCKS * P

    if len(used) <= tot:
        # compact path: transform only referenced rows
        used_pad = np.empty(tot, dtype=np.int64)
        used_pad[:len(used)] = used
        used_pad[len(used):] = int(used[-1]) if len(used) else 0
        compact = c2e_weight[used_pad].astype(BF16)   # [tot, 128]

        in_maps = []
        for i in range(N_CORES):
            sl = compact[i * rows:(i + 1) * rows]
            in_maps.append({"tslice": np.ascontiguousarray(
                np.concatenate([prefix, sl.T], axis=1))})

        res = _run(_get_nc(BLOCKS), in_maps)

        t2u = np.concatenate(
            [np.asarray(res.results[i]["out"]).T for i in range(N_CORES)],
            axis=0)                                   # [tot, 128] bf16
        return t2u[inv].astype(np.float32)

    # fallback: transform the whole vocab with staggered windows
    frows = FULL_BLOCKS * P
    tableT = np.ascontiguousarray(c2e_weight.T).astype(BF16)  # [128, vocab]
    starts, in_maps = [], []
    rng = vocab // N_CORES
    for i in range(N_CORES):
        start = min(i * rng, vocab - frows)
        starts.append(start)
        in_maps.append({"tslice": np.ascontiguousarray(
            np.concatenate([prefix, tableT[:, start:start + frows]], axis=1))})

    res = _run(_get_nc(FULL_BLOCKS), in_maps)

    t2 = np.empty((vocab, D), dtype=np.float32)
    for i in range(N_CORES):
        dense = np.asarray(res.results[i]["out"])     # [128, frows]
        lo = i * rng
        hi = min((i + 1) * rng, vocab)
        t2[lo:hi] = dense[:, lo - starts[i]:hi - starts[i]].T
    return t2[nodes]
